# revision 21
# baseline (speedup 1.0000x reference)
"""GPT forward (6-layer, E=768, H=12, N=1024, B=2, V=50257) on 8 TRN2 cores.

Sharding: sequence-sharded layers (cores 0-3 batch 0, cores 4-7 batch 1;
core in-group index g owns row-blocks g and 7-g of its batch), vocab-sharded
lm_head (each core computes all 2048 rows x a 6283-wide vocab slice).

Per-layer schedule keeps the PE array dense (HAM clock gate) and hides the
K/V AllGather latency with a row-block pipeline:
  FFN2(l-1) both rbs (w2 streamed once, kf-outer)
  LN1(rb0) -> KV-proj(rb0) -> ship CC-A (key blocks 0-3)
  LN1(rb1) -> KV-proj(rb1) -> ship CC-B (key blocks 4-7)
  Q-proj -> [CC-A lands] S/AV/out_proj/LN2 for query-block 0
         -> [CC-B lands] S/AV/out_proj/LN2 for query-block 1 -> FFN1
CC-A gates only the qb0 attention path; CC-B's flight hides under it.
V/out_proj/FFN2 run activation-stationary with 768-wide free dims; V comes
out directly in [rows, feat] layout (no V transposes). Softmax denominators
come from an appended ones-column on V; the causal mask is multiplicative
bf16 applied after exp (scores are small so max-subtraction is skipped).

Compute: bf16 matmul inputs (K shipped as fp8), fp32 PSUM/residual/LN.
LN gammas folded into following weights host-side, betas into biases.
Logits are written bf16 and upcast on host.
"""

from contextlib import ExitStack

import numpy as np
import ml_dtypes

import concourse.bass as bass
import concourse.mybir as mybir
import concourse.tile as tile
from concourse.bass_utils import run_bass_kernel_spmd
from concourse.vector_clock import ScopedClock, VectorClock

F32 = mybir.dt.float32
BF16 = mybir.dt.bfloat16
AF = mybir.ActivationFunctionType
BF = ml_dtypes.bfloat16
FP8 = mybir.dt.float8e4

V, E, N, H, L = 50257, 768, 1024, 12, 6
HD = E // H          # 64
FF = 4 * E           # 3072
B = 2
KT = E // 128        # 6 feature k-tiles
MT_QK = 2 * KT       # 12 m-tiles for fused q,k
MT_FF = FF // 128    # 24
R = 256              # rows per core
NB = 8               # 128-row blocks per batch
H65 = H * 65         # 780: per-row V layout incl ones columns
VP = 6283            # vocab slice per core (8*6283 = 50264 >= V)
VPAD = 13 * 512      # host-side padded vocab slice (6656)
NCH = 13             # head vocab chunks of 512 (last used width = 139)
KV_GROUPS = [[0, 1, 2, 3], [4, 5, 6, 7]]
H8_GROUPS = [[0, 1, 2, 3, 4, 5, 6, 7]]
CC_SZ = E * 128 + 2 * 128 * H65  # bytes: K fp8 + V bf16, one row-block
EPS = 1e-5
A_RTS = [0, 1, 2, 3, 8, 9, 10, 11]       # row-tiles from CC-H-A (rb0 blocks)
B_RTS = [7, 6, 5, 4, 15, 14, 13, 12]     # row-tiles from CC-H-B


class _TileContext(tile.TileContext):
    """This image's walrus rejects Drain instructions with >1 sync-wait.
    Split the kernel-tail drain into one Drain per pending proc."""

    def _drain_and_barrier(self, tick_clock, wait_clock):
        nc = self.nc
        vec = tick_clock.global_clock
        n = len(vec)
        for proc in range(n):
            t = vec[proc]
            if t <= 0:
                continue
            sub = VectorClock([t if i == proc else 0 for i in range(n)])
            d = nc.sync.drain()
            wait_clock.add_sem_waits(d.ins, ScopedClock({None: sub}))
        nc.sync.drain()
        nc.all_engine_barrier()
        assert self.sems is not None
        popped = nc._tile_sem_poison_stack.pop()
        assert popped is self._sem_poison
        nc.clear_and_free_semaphores(list(self.sems.allocated().values()))
        nc.all_engine_barrier()


def _split_multi_waits(nc):
    """This walrus build encodes at most one sync-wait per instruction.
    Hoist extra waits onto NoOps inserted just before, on the same engine."""
    ctr = 0
    for bb in nc.main_func.blocks:
        il = bb.instructions
        out_l = []
        for ins in il:
            si = ins.sync_info
            if si is not None and si.on_wait is not None and len(si.on_wait) > 1:
                waits = list(si.on_wait)
                for w in waits[:-1]:
                    noop = mybir.InstNoOp(name=f"wsplit_{ctr}", ins=[], outs=[])
                    ctr += 1
                    noop.engine = ins.engine
                    noop.sync_info = type(si)(on_wait=[w], on_update=[])
                    out_l.append(noop)
                si.on_wait = waits[-1:]
            out_l.append(ins)
        il[:] = out_l


def _bcast_row(t, row, p=128):
    """AP reading DRAM row `t[row]` broadcast across p partitions."""
    base = t[row] if row is not None else t[:]
    return bass.AP(
        tensor=base.tensor, offset=base.offset,
        ap=[[0, p]] + [list(x) for x in base.ap])


def build_nc(use_bo, use_b2, use_bv):
    nc = bass.Bass(num_devices=8)

    h0_in = nc.declare_dram_parameter("h0", [2, 128, E], F32, isOutput=False)
    # pre-tiled: [L, 12, 128, KT*128] (m-tile, partition=feat%128, kt*128+col)
    wqk_in = nc.declare_dram_parameter("wqk", [L, MT_QK, 128, E], BF16, isOutput=False)
    bqk_in = nc.declare_dram_parameter("bqk", [L, 2 * E], F32, isOutput=False)
    wv_in = nc.declare_dram_parameter("wv", [L, E, E], BF16, isOutput=False)
    wo_in = nc.declare_dram_parameter("wo", [L, E, E], BF16, isOutput=False)
    w1_in = nc.declare_dram_parameter("w1", [L, MT_FF, 128, E], BF16, isOutput=False)
    b1_in = nc.declare_dram_parameter("b1", [L, FF], F32, isOutput=False)
    w2_in = nc.declare_dram_parameter("w2", [L, FF, E], BF16, isOutput=False)
    mask_in = nc.declare_dram_parameter("masks", [NB, 128, 256], BF16, isOutput=False)
    # pre-tiled: [NCH, 128, KT*512]
    wh_in = nc.declare_dram_parameter("whead", [NCH, 128, KT * 512], BF16, isOutput=False)
    ident_in = nc.declare_dram_parameter("ident", [128, 128], BF16, isOutput=False)
    bo_in = nc.declare_dram_parameter("bo", [L, E], F32, isOutput=False) if use_bo else None
    b2_in = nc.declare_dram_parameter("b2", [L, E], F32, isOutput=False) if use_b2 else None
    bv_in = nc.declare_dram_parameter("bv", [L, H65], F32, isOutput=False) if use_bv else None
    out = nc.declare_dram_parameter("logits", [B * N, VP], BF16, isOutput=True)

    cc_in = [[nc.dram_tensor(f"cc_i{l}_{rb}", [CC_SZ], FP8) for rb in range(2)]
             for l in range(L)]
    cc_out = [[nc.dram_tensor(f"cc_o{l}_{rb}", [4, CC_SZ], FP8)
               for rb in range(2)] for l in range(L)]
    cc_h_in = [nc.dram_tensor(f"cch_i{rb}", [E * 128], BF16) for rb in range(2)]
    cc_h_out = [nc.dram_tensor(f"cch_o{rb}", [8, E * 128], BF16, addr_space="Shared")
                for rb in range(2)]

    with _TileContext(nc) as tc, ExitStack() as ctx:
        const = ctx.enter_context(tc.tile_pool(name="const", bufs=1))

        ident = const.tile([128, 128], BF16)
        nc.sync.dma_start(out=ident, in_=ident_in[:])

        h_sb = [const.tile([128, E], F32, tag=f"h{rb}", name=f"h{rb}") for rb in range(2)]
        for rb in range(2):
            nc.sync.dma_start(out=h_sb[rb], in_=h0_in[rb])

        mask_sb = const.tile([128, NB, 256], BF16)
        nc.sync.dma_start(out=mask_sb, in_=mask_in.rearrange("k p c -> p k c"))

        hfT_all = const.tile([128, 16, KT, 128], BF16, tag="hfT_all")

        eps_t = const.tile([128, 1], F32)
        nc.vector.memset(eps_t, EPS)

        with ExitStack() as lctx:
            p = {
                "small": lctx.enter_context(tc.tile_pool(name="small", bufs=2)),
                "work": lctx.enter_context(tc.tile_pool(name="work", bufs=2)),
                "work1": lctx.enter_context(tc.tile_pool(name="work1", bufs=1)),
                "wqkp": lctx.enter_context(tc.tile_pool(name="wqkp", bufs=6)),
                "wvp": lctx.enter_context(tc.tile_pool(name="wvp", bufs=6)),
                "wop": lctx.enter_context(tc.tile_pool(name="wop", bufs=6)),
                "wstream": lctx.enter_context(tc.tile_pool(name="wstream", bufs=2)),
                "big": lctx.enter_context(tc.tile_pool(name="big", bufs=1)),
                "eSp": lctx.enter_context(tc.tile_pool(name="eSp", bufs=13)),
                "ps256": lctx.enter_context(
                    tc.tile_pool(name="ps256", bufs=3, space="PSUM")),
                "psO": lctx.enter_context(
                    tc.tile_pool(name="psO", bufs=1, space="PSUM")),
                "psW": lctx.enter_context(
                    tc.tile_pool(name="psW", bufs=2, space="PSUM")),
            }

            def layernorm_rb(rb, xT_dst):
                """LN of h_sb[rb] -> bf16 y -> 6 transposes into
                xT_dst[:, :, rb*128:(rb+1)*128]."""
                mv = p["small"].tile([128, nc.vector.BN_AGGR_DIM], F32, tag="ln_mv")
                stats = p["small"].tile(
                    [128, 3, nc.vector.BN_STATS_DIM], F32, tag="ln_st")
                xin = h_sb[rb]
                for s in range(3):
                    nc.vector.bn_stats(
                        out=stats[:, s, :], in_=xin[:, s * 256:(s + 1) * 256])
                nc.vector.bn_aggr(out=mv, in_=stats)
                rstd = p["small"].tile([128, 1], F32, tag="ln_rstd")
                nc.scalar.activation(
                    out=rstd, in_=mv[:, 1:2], func=AF.Sqrt, bias=eps_t, scale=1.0)
                nc.vector.reciprocal(out=rstd, in_=rstd)
                y = p["work"].tile([128, E], BF16, tag="ln_y")
                nc.vector.tensor_scalar(
                    out=y, in0=xin, scalar1=mv[:, 0:1], scalar2=rstd,
                    op0=mybir.AluOpType.subtract, op1=mybir.AluOpType.mult)
                for kt in range(KT):
                    pst = p["ps256"].tile([128, 128], BF16, tag="mm256",
                                          name=f"pst_{rb}_{kt}")
                    nc.tensor.transpose(pst, y[:, kt * 128:(kt + 1) * 128], ident)
                    nc.vector.tensor_copy(
                        out=xT_dst[:, kt, rb * 128:(rb + 1) * 128], in_=pst)

            def ffn2_layer(l, gT, b2_b):
                """FFN2 of layer l, both row-blocks, w2 streamed once
                (grouped loads of 4 row-tiles), 768-wide matmuls."""
                psw = [p["psW"].tile([128, E], F32, tag="psW",
                                     name=f"psw2_{l}_{rb}") for rb in range(2)]
                for kg in range(MT_FF // 4):
                    w2_g = p["wstream"].tile([128, 4, E], BF16, tag="w2_g")
                    nc.sync.dma_start(
                        out=w2_g,
                        in_=w2_in[l, kg * 512:(kg + 1) * 512, :].rearrange(
                            "(m q) e -> q m e", q=128))
                    for mm in range(4):
                        kf = kg * 4 + mm
                        for rb in range(2):
                            for lo, hi in ((0, 512), (512, E)):
                                nc.tensor.matmul(
                                    psw[rb][:, lo:hi],
                                    gT[:, kf, rb * 128:(rb + 1) * 128],
                                    w2_g[:, mm, lo:hi],
                                    start=(kf == 0), stop=(kf == MT_FF - 1))
                for rb in range(2):
                    nc.vector.tensor_add(out=h_sb[rb], in0=h_sb[rb], in1=psw[rb])
                    if b2_b is not None:
                        nc.vector.tensor_add(out=h_sb[rb], in0=h_sb[rb], in1=b2_b)

            for l in range(L):
                if l > 0:
                    b2_b = None
                    if b2_in is not None:
                        b2_b = p["small"].tile([128, E], F32, tag="b2_b")
                        nc.sync.dma_start(out=b2_b, in_=_bcast_row(b2_in, l - 1))
                    ffn2_layer(l - 1, gT, b2_b)  # noqa: F821

                xT = p["work"].tile([128, KT, R], BF16, tag="xT")
                kT_c = p["work1"].tile([128, KT, R], FP8, tag="kT_c")
                va_c = p["work1"].tile([128, 2, H65], BF16, tag="va_c")
                nc.vector.memset(
                    va_c.rearrange("q a (h o) -> q a h o", o=65)[:, :, :, 64:65], 1.0)

                bq = p["small"].tile([128, MT_QK], F32, tag="bqk")
                nc.sync.dma_start(
                    out=bq, in_=bqk_in[l].rearrange("(m q) -> q m", q=128))
                bv_b = None
                if bv_in is not None:
                    bv_b = p["small"].tile([128, H65], F32, tag="bv_b")
                    nc.sync.dma_start(out=bv_b, in_=_bcast_row(bv_in, l))

                wqk_t = {}   # K m-tiles kept across rbs
                wv_t = {}    # V weight slices kept across rbs

                def kq_mtile(m, dst_ap, col, wcache):
                    """One 128-wide output m-tile of the q/k projection."""
                    if wcache is not None and m in wcache:
                        w_m = wcache[m]
                    else:
                        w_m = p["wqkp"].tile([128, KT, 128], BF16, tag="wqk_m")
                        nc.sync.dma_start(
                            out=w_m,
                            in_=wqk_in[l, m].rearrange("q (kt c) -> q kt c", kt=KT))
                        if wcache is not None:
                            wcache[m] = w_m
                    ps = p["ps256"].tile([128, R], F32, tag="mm256")
                    wN = 128 if col is not None else R
                    src = (xT[:, :, col * 128:(col + 1) * 128]
                           if col is not None else xT)
                    for kt in range(KT):
                        nc.tensor.matmul(ps[:, 0:wN], w_m[:, kt, :], src[:, kt, :],
                                         start=(kt == 0), stop=(kt == KT - 1))
                    nc.vector.tensor_scalar_add(
                        out=dst_ap, in0=ps[:, 0:wN], scalar1=bq[:, m:m + 1])

                for rb in range(2):
                    layernorm_rb(rb, xT)
                    # K m-tiles for this row-block (m 6..11 of wqk)
                    for mk in range(KT):
                        kq_mtile(KT + mk, kT_c[:, mk, rb * 128:(rb + 1) * 128],
                                 rb, wqk_t)
                    # V projection, activation-stationary: out [rows, feat]
                    psv = p["psW"].tile([128, E], F32, tag="psW",
                                        name=f"psv_{l}_{rb}")
                    for kt in range(KT):
                        if rb == 0:
                            wvt = p["wvp"].tile([128, E], BF16, tag="wv_t")
                            nc.sync.dma_start(
                                out=wvt, in_=wv_in[l, kt * 128:(kt + 1) * 128, :])
                            wv_t[kt] = wvt
                        for lo, hi in ((0, 512), (512, E)):
                            nc.tensor.matmul(
                                psv[:, lo:hi],
                                xT[:, kt, rb * 128:(rb + 1) * 128],
                                wv_t[kt][:, lo:hi],
                                start=(kt == 0), stop=(kt == KT - 1))
                    vav = va_c.rearrange("q a (h o) -> q a h o", o=65)
                    src = psv.rearrange("q (h o) -> q h o", o=64)
                    if bv_b is not None:
                        bsrc = bv_b.rearrange("q (h o) -> q h o", o=65)
                        nc.vector.tensor_add(
                            out=vav[:, rb, :, 0:64], in0=src,
                            in1=bsrc[:, :, 0:64])
                    else:
                        nc.vector.tensor_copy(out=vav[:, rb, :, 0:64], in_=src)
                    # ship this row-block's K+V (K as (q, kt, c) for 768B lines)
                    nc.sync.dma_start(
                        out=cc_in[l][rb][0:E * 128].rearrange(
                            "(q x) -> q x", q=128),
                        in_=kT_c[:, :, rb * 128:(rb + 1) * 128])
                    nc.sync.dma_start(
                        out=cc_in[l][rb][E * 128:].bitcast(BF16).rearrange(
                            "(q x) -> q x", q=128),
                        in_=va_c[:, rb, :])
                    nc.gpsimd.collective_compute(
                        "AllGather", mybir.AluOpType.bypass,
                        replica_groups=KV_GROUPS,
                        ins=[cc_in[l][rb][:]], outs=[cc_out[l][rb][:]])

                # Q m-tiles (m 0..5), both row-blocks, while gathers fly
                qT = p["work1"].tile([128, KT, R], BF16, tag="qT")
                for mq in range(KT):
                    kq_mtile(mq, qT[:, mq, :], None, None)



                # ---- gathered K/V landing zones ----
                kT_all = p["big"].tile([128, NB, KT, 128], FP8, tag="kT_all")
                V_all = p["big"].tile([128, NB, H65], BF16, tag="V_all")

                def pull_cc(rb):
                    kview = cc_out[l][rb][:, 0:E * 128].rearrange(
                        "g (q x) -> g q x", q=128)
                    vview = cc_out[l][rb][:, E * 128:].bitcast(BF16).rearrange(
                        "g (q x) -> g q x", q=128)
                    for g in range(4):
                        kb = g if rb == 0 else 7 - g
                        nc.sync.dma_start(
                            out=kT_all[:, kb, :, :].rearrange("q kt c -> q (kt c)"),
                            in_=kview[g])
                        nc.sync.dma_start(out=V_all[:, kb, :], in_=vview[g])

                # ---- attention ----
                o_pack = p["work1"].tile([128, 2, E], BF16, tag="o_pack")
                eS = [p["eSp"].tile([128, 4 * 256 + 4 * 128], BF16, tag="eS",
                                    name=f"eS_{l}_{hh}") for hh in range(H)]

                def emit_S(hh, kb):
                    par = hh % 2
                    wN = 256 if kb < 4 else 128
                    off = kb * 256 if kb < 4 else 1024 + (kb - 4) * 128
                    rhs_q = qT[par * 64:par * 64 + 64, hh // 2, :]
                    ps = p["ps256"].tile([128, R], F32, tag="mm256")
                    nc.tensor.matmul(
                        ps[:, 0:wN],
                        kT_all[par * 64:par * 64 + 64, kb, hh // 2, :],
                        rhs_q if kb < 4 else rhs_q[:, 128:256],
                        start=True, stop=True)
                    nc.scalar.activation(
                        out=eS[hh][:, off:off + wN], in_=ps[:, 0:wN], func=AF.Exp)
                    m_sl = (mask_sb[:, kb, 0:256] if kb < 4
                            else mask_sb[:, kb, 128:256])
                    nc.vector.tensor_mul(
                        out=eS[hh][:, off:off + wN],
                        in0=eS[hh][:, off:off + wN], in1=m_sl)

                def emit_AV(hh, qb):
                    nkb = 4 if qb == 0 else NB
                    psO = p["psO"].tile([128, 65], F32, tag="psO")
                    for kb in range(nkb):
                        if kb < 4:
                            sl = eS[hh][:, kb * 256 + qb * 128:
                                        kb * 256 + qb * 128 + 128]
                        else:
                            sl = eS[hh][:, 1024 + (kb - 4) * 128:
                                        1024 + (kb - 4) * 128 + 128]
                        nc.tensor.matmul(
                            psO, sl, V_all[:, kb, hh * 65:(hh + 1) * 65],
                            start=(kb == 0), stop=(kb == nkb - 1))
                    recip = p["small"].tile([128, 1], F32, tag="recip")
                    nc.vector.reciprocal(out=recip, in_=psO[:, 64:65])
                    nc.vector.tensor_scalar_mul(
                        out=o_pack[:, qb, hh * 64:(hh + 1) * 64],
                        in0=psO[:, 0:64], scalar1=recip)

                def attn_path(qb, oT, x2T, bo_b, wo_t):
                    """S for key blocks of cc rb=qb, AV one head-pair behind,
                    o-transpose + out_proj accumulation one pair behind AV —
                    the whole path pipelines with no serial tail."""
                    kbs = range(4) if qb == 0 else range(4, 8)
                    psr = p["psW"].tile([128, E], F32, tag="psW",
                                        name=f"psra_{l}_{qb}")

                    def trans_op(f):
                        pst = p["ps256"].tile([128, 128], BF16, tag="mm256",
                                              name=f"psto_{l}_{qb}_{f}")
                        nc.tensor.transpose(
                            pst, o_pack[:, qb, f * 128:(f + 1) * 128], ident)
                        nc.vector.tensor_copy(
                            out=oT[:, f, qb * 128:(qb + 1) * 128], in_=pst)
                        if qb == 0:
                            wot = p["wop"].tile([128, E], BF16, tag="wo_t")
                            nc.sync.dma_start(
                                out=wot, in_=wo_in[l, f * 128:(f + 1) * 128, :])
                            wo_t[f] = wot
                        for lo, hi in ((0, 512), (512, E)):
                            nc.tensor.matmul(
                                psr[:, lo:hi],
                                oT[:, f, qb * 128:(qb + 1) * 128],
                                wo_t[f][:, lo:hi],
                                start=(f == 0), stop=(f == KT - 1))

                    for hp in range(H // 2):
                        for kb in kbs:
                            emit_S(2 * hp, kb)
                            emit_S(2 * hp + 1, kb)
                        if hp >= 1:
                            emit_AV(2 * (hp - 1), qb)
                            emit_AV(2 * (hp - 1) + 1, qb)
                        if hp >= 2:
                            trans_op(hp - 2)
                    for hh in range(H - 2, H):
                        emit_AV(hh, qb)
                    for f in range(KT - 2, KT):
                        trans_op(f)
                    nc.vector.tensor_add(out=h_sb[qb], in0=h_sb[qb], in1=psr)
                    if bo_b is not None:
                        nc.vector.tensor_add(out=h_sb[qb], in0=h_sb[qb], in1=bo_b)
                    layernorm_rb(qb, x2T)

                bo_b = None
                if bo_in is not None:
                    bo_b = p["small"].tile([128, E], F32, tag="bo_b")
                    nc.sync.dma_start(out=bo_b, in_=_bcast_row(bo_in, l))
                oT = p["work1"].tile([128, KT, R], BF16, tag="oT")
                x2T = p["work1"].tile([128, KT, R], BF16, tag="x2T")
                wo_t = {}

                pull_cc(0)
                pull_cc(1)
                attn_path(0, oT, x2T, bo_b, wo_t)
                attn_path(1, oT, x2T, bo_b, wo_t)

                # ---- FFN1 (gelu+bias at evict), both row-blocks ----
                b1s = p["small"].tile([128, MT_FF], F32, tag="b1s")
                nc.sync.dma_start(
                    out=b1s, in_=b1_in[l].rearrange("(m q) -> q m", q=128))
                gT = p["big"].tile([128, MT_FF, R], BF16, tag="gT")
                for mg in range(MT_FF // 4):
                    w1_g = p["wstream"].tile([128, 4, KT, 128], BF16, tag="w1_g")
                    nc.sync.dma_start(
                        out=w1_g,
                        in_=w1_in[l, 4 * mg:4 * mg + 4].rearrange(
                            "m q (kt c) -> q m kt c", kt=KT))
                    for mm in range(4):
                        m = 4 * mg + mm
                        ps = p["ps256"].tile([128, R], F32, tag="mm256")
                        for kt in range(KT):
                            nc.tensor.matmul(ps, w1_g[:, mm, kt, :], x2T[:, kt, :],
                                             start=(kt == 0), stop=(kt == KT - 1))
                        nc.scalar.activation(
                            out=gT[:, m, :], in_=ps, func=AF.Gelu_apprx_tanh,
                            bias=b1s[:, m:m + 1], scale=1.0)

            # ---- final: FFN2 + LN per row-block, ship gathers ----
            b2_b = None
            if b2_in is not None:
                b2_b = p["small"].tile([128, E], F32, tag="b2_b")
                nc.sync.dma_start(out=b2_b, in_=_bcast_row(b2_in, L - 1))
            ffn2_layer(L - 1, gT, b2_b)
            hfT = p["work"].tile([128, KT, R], BF16, tag="xT")
            for rb in range(2):
                layernorm_rb(rb, hfT)
                nc.sync.dma_start(
                    out=cc_h_in[rb][:].rearrange("(q x) -> q x", q=128),
                    in_=hfT[:, :, rb * 128:(rb + 1) * 128])
                nc.gpsimd.collective_compute(
                    "AllGather", mybir.AluOpType.bypass, replica_groups=H8_GROUPS,
                    ins=[cc_h_in[rb][:]], outs=[cc_h_out[rb][:]])
            for rb, rts in ((0, A_RTS), (1, B_RTS)):
                hgv = cc_h_out[rb].rearrange("g (q x) -> g q x", q=128)
                for g in range(8):
                    # slot g of gather rb = core g's row-block rb
                    rt = rts[g]
                    nc.sync.dma_start(
                        out=hfT_all[:, rt, :, :].rearrange("q kt c -> q (kt c)"),
                        in_=hgv[g])


        # ---- lm_head ----
        with ExitStack() as hctx:
            whp = hctx.enter_context(tc.tile_pool(name="whp", bufs=3))
            lsb = hctx.enter_context(tc.tile_pool(name="lsb", bufs=6))
            psH = hctx.enter_context(tc.tile_pool(name="psH", bufs=6, space="PSUM"))
            for nch in range(NCH):
                wN = 512 if nch < NCH - 1 else VP - 512 * (NCH - 1)
                wh = whp.tile([128, KT, 512], BF16, tag="wh")
                nc.sync.dma_start(
                    out=wh,
                    in_=wh_in[nch].rearrange("q (kt c) -> q kt c", kt=KT))
                rts = (A_RTS + B_RTS) if nch == 0 else list(range(16))
                for i, rt in enumerate(rts):
                    ps = psH.tile([128, 512], F32, tag="psH")
                    for kt in range(KT):
                        nc.tensor.matmul(
                            ps[:, 0:wN], hfT_all[:, rt, kt, :],
                            wh[:, kt, 0:wN], start=(kt == 0), stop=(kt == KT - 1))
                    ls = lsb.tile([128, 512], BF16, tag="ls")
                    if i % 2 == 0:
                        nc.vector.tensor_copy(out=ls[:, 0:wN], in_=ps[:, 0:wN])
                    else:
                        nc.scalar.activation(
                            out=ls[:, 0:wN], in_=ps[:, 0:wN], func=AF.Copy)
                    nc.sync.dma_start(
                        out=out[rt * 128:(rt + 1) * 128, nch * 512:nch * 512 + wN],
                        in_=ls[:, 0:wN])
    _split_multi_waits(nc)
    return nc


# ---------------------------------------------------------------------------
# host side
# ---------------------------------------------------------------------------

def _sinusoidal_pos(n, dim):
    pos = np.arange(n, dtype=np.float32)[:, None]
    i = np.arange(0, dim, 2, dtype=np.float32)
    j = np.arange(1, dim, 2, dtype=np.float32)
    s = np.sin(pos / np.power(np.float32(10000.0), 2.0 * i / dim, dtype=np.float32))
    c = np.cos(pos / np.power(np.float32(10000.0), 2.0 * j / dim, dtype=np.float32))
    return np.stack([s, c], axis=-1).reshape(n, dim).astype(np.float32)


_CACHE = {}


def _get_nc(use_bo, use_b2, use_bv):
    key = (use_bo, use_b2, use_bv)
    if key not in _CACHE:
        _CACHE[key] = build_nc(use_bo, use_b2, use_bv)
    return _CACHE[key]


def _tile_w(w):
    """[E, M*128] -> [M, 128, KT*128]: [m, p, kt*128+c] = w[kt*128+p, m*128+c]."""
    M = w.shape[1] // 128
    return np.ascontiguousarray(
        w.reshape(KT, 128, M, 128).transpose(2, 1, 0, 3).reshape(M, 128, KT * 128))


def kernel(x, tok_emb, wq, wk, wv, wo, bo, ln1_g, ln1_b, ln2_g, ln2_b,
           w1, b1, w2, b2, lnf_g, lnf_b, w_head, _trace=False):
    x = np.asarray(x)
    f = lambda a: np.asarray(a, dtype=np.float32)
    tok_emb, wq, wk, wv, wo = f(tok_emb), f(wq), f(wk), f(wv), f(wo)
    bo, w1, b1, w2, b2 = f(bo), f(w1), f(b1), f(w2), f(b2)
    ln1_g, ln1_b, ln2_g, ln2_b = f(ln1_g), f(ln1_b), f(ln2_g), f(ln2_b)
    lnf_g, lnf_b, w_head = f(lnf_g), f(lnf_b), f(w_head)

    h0 = tok_emb[x] + _sinusoidal_pos(N, E)[None, :, :]     # [B, N, E] f32

    scale = np.float32(1.0 / np.sqrt(HD))
    wqk = np.concatenate([wq * scale, wk], axis=2)           # [L, E, 2E]
    bqk = np.einsum("le,lef->lf", ln1_b, wqk).astype(np.float32)
    wqk = (ln1_g[:, :, None] * wqk).astype(BF)
    wqk_t = np.stack([_tile_w(wqk[l]) for l in range(L)])
    wvf = np.ascontiguousarray((ln1_g[:, :, None] * wv).astype(BF))
    bv = np.einsum("le,lef->lf", ln1_b, wv).astype(np.float32)  # [L, E]
    bv_vc = np.zeros((L, H65), dtype=np.float32)
    bv_vc.reshape(L, H, 65)[:, :, 0:64] = bv.reshape(L, H, 64)
    b1c = (b1 + np.einsum("le,lef->lf", ln2_b, w1)).astype(np.float32)
    w1f = (ln2_g[:, :, None] * w1).astype(BF)
    w1_t = np.stack([_tile_w(w1f[l]) for l in range(L)])
    w2f = np.ascontiguousarray(w2.astype(BF))
    wof = np.ascontiguousarray(wo.astype(BF))
    whf = np.zeros((E, 8 * VPAD), dtype=np.float32)
    wh_scaled = lnf_g[:, None] * w_head
    for c in range(8):
        lo, hi = c * VP, min((c + 1) * VP, V)
        whf[:, c * VPAD:c * VPAD + (hi - lo)] = wh_scaled[:, lo:hi]
    whf = whf.astype(BF)

    use_bo = bool(np.any(bo))
    use_b2 = bool(np.any(b2))
    use_bv = bool(np.any(bv))
    nc = _get_nc(use_bo, use_b2, use_bv)

    ident = np.eye(128, dtype=BF)
    key_idx = np.arange(N)[:, None]
    in_maps = []
    for c in range(8):
        bb, g = c // 4, c % 4
        blocks = [g, 7 - g]
        h0c = np.stack([h0[bb, blk * 128:(blk + 1) * 128, :] for blk in blocks])
        masks = np.zeros((NB, 128, 256), dtype=BF)
        for qi, blk in enumerate(blocks):
            q = blk * 128 + np.arange(128)[None, :]
            allow = (key_idx <= q).astype(np.float32).reshape(NB, 128, 128)
            masks[:, :, qi * 128:(qi + 1) * 128] = allow.astype(BF)
        # whead slice, re-tiled to [NCH, 128, KT*512]
        whc = whf[:, c * VPAD:(c + 1) * VPAD]
        whc_t = np.ascontiguousarray(
            whc.reshape(KT, 128, NCH, 512).transpose(2, 1, 0, 3).reshape(
                NCH, 128, KT * 512))
        m = {
            "h0": np.ascontiguousarray(h0c, dtype=np.float32),
            "wqk": wqk_t, "bqk": bqk, "wv": wvf, "wo": wof,
            "w1": w1_t, "b1": b1c, "w2": w2f,
            "masks": masks, "whead": whc_t, "ident": ident,
        }
        if use_bo:
            m["bo"] = bo
        if use_b2:
            m["b2"] = b2
        if use_bv:
            m["bv"] = bv_vc
        in_maps.append(m)

    res = run_bass_kernel_spmd(nc, in_maps, list(range(8)), trace=_trace)
    logits = np.concatenate(
        [res.results[c]["logits"].astype(np.float32) for c in range(8)], axis=1)
    logits = logits[:, :V]
    if np.any(lnf_b):
        logits = logits + (lnf_b @ w_head)[None, :]
    out = logits.reshape(B, N, V)
    if _trace:
        return out, res
    return out
